# revision 28
# baseline (speedup 1.0000x reference)
"""Trainium2 Bass kernel: NF4 (bitsandbytes-style) dequant + linear.

y = x @ dequant(weight_q, absmax).T + bias

x:        [4, 2048, 4096] f32
weight_q: [11008, 4096] int32 (values 0..15, NF4 codes)
absmax:   [11008, 64] f32 (per-64-block scales)
bias:     [11008] f32
out:      [4, 2048, 11008] f32

Sharding: column-parallel over out_features across 8 cores (1376 each).
Compute in bf16 with f32 PSUM accumulation. Host prep dequantizes the
weight (16-entry NF4 code lookup × per-64-block absmax) to bf16 and lays
it out contraction-major [128, KT*O_LOC]; x is transposed to [4096,
8192] bf16 so the contraction dim is on SBUF partitions.

Device schedule: weights stream in via 8 large DMAs (4 k-tiles each) in
exactly the order the first token tile consumes them, so the PE starts
within a few µs and never waits on weight delivery after the first
token tile. The matmul loop is kt-outer / o-chunk-inner: each stationary
(x) tile feeds 3 matmuls (512+512+352 = 1376 moving columns per
128-token stationary load). PSUM accumulates over the 32 k-tiles;
eviction adds the bias on DVE and stores via the scalar-engine ring.
"""

import numpy as np
import ml_dtypes

import concourse.bacc as bacc
import concourse.mybir as mybir
import concourse.tile as tile
from concourse.alu_op_type import AluOpType
from concourse.bass_utils import run_bass_kernel_spmd

DT = mybir.dt

NF4 = np.array([
    -1.0, -0.6961928009986877, -0.5250730514526367, -0.39491748809814453,
    -0.28444138169288635, -0.18477343022823334, -0.09105003625154495, 0.0,
    0.07958029955625534, 0.16093020141124725, 0.24611230194568634,
    0.33791524171829224, 0.44070982933044434, 0.5626170039176941,
    0.7229568362236023, 1.0], dtype=np.float32)

P = 128
IN_F = 4096
OUT_F = 11008
N_CORES = 8
O_LOC = OUT_F // N_CORES          # 1376 out features per core
S_TOT = 4 * 2048                  # 8192 tokens
KT = IN_F // P                    # 32 contraction tiles
SP = 256                          # tokens per x macro tile (2 psum tiles)
NSP = S_TOT // SP                 # 32 x macro tiles
O_CHUNKS = [(0, 512), (512, 512), (1024, 352)]
STRIP_MM_UPDATES = True

_CACHE = {}


def _build():
    nc = bacc.Bacc()
    # x host-blocked so every x-tile DMA is fully contiguous per
    # partition: xH[p, sp, kt, s] = x[sp*SP+s, kt*128+p]
    xH = nc.dram_tensor("xH", [P, NSP * KT * SP], DT.bfloat16,
                        kind="ExternalInput")
    w_d = nc.dram_tensor("wd", [P, KT * O_LOC], DT.bfloat16,
                         kind="ExternalInput")
    biasb = nc.dram_tensor("biasb", [1, O_LOC], DT.bfloat16,
                           kind="ExternalInput")
    y = nc.dram_tensor("y", [S_TOT, O_LOC], DT.float32, kind="ExternalOutput")

    with tile.TileContext(nc) as tc:
        with (
            tc.tile_pool(name="w", bufs=1) as wpool,
            tc.tile_pool(name="x", bufs=3) as xp,
            tc.tile_pool(name="o", bufs=4) as op,
            tc.tile_pool(name="ps", bufs=8, space="PSUM") as psp,
            tc.tile_pool(name="c", bufs=1) as cst,
        ):
            def load_x(sp):
                c0 = sp * KT * SP
                xb = xp.tile([P, KT, SP], DT.bfloat16, tag="xb", name="xb")
                nc.sync.dma_start(
                    out=xb[:], in_=xH[:, c0:c0 + KT * SP]
                    .rearrange("p (k s) -> p k s", k=KT))
                return xb

            wf = {}           # kt -> (tile, row within tile)

            def load_w(g, k0, kg, eng=None):
                w_t = wpool.tile([P, kg, O_LOC], DT.bfloat16, tag=f"wf_{g}")
                (eng or nc.sync).dma_start(
                    out=w_t[:],
                    in_=w_d[:, k0 * O_LOC:(k0 + kg) * O_LOC]
                        .rearrange("p (k o) -> p k o", k=kg))
                for k in range(kg):
                    wf[k0 + k] = (w_t, k)

            # ---- startup: x(0) in 4 pieces interleaved with graded w
            # chunks, ordered so neither PE operand stream starves ----
            xb0 = xp.tile([P, KT, SP], DT.bfloat16, tag="xb", name="xb")

            def load_x0_piece(g):
                nc.sync.dma_start(
                    out=xb0[:, g * 8:(g + 1) * 8, :],
                    in_=xH[:, g * 8 * SP:(g + 1) * 8 * SP]
                        .rearrange("p (k s) -> p k s", k=8))

            load_x0_piece(0)
            load_w(0, 0, 1)
            load_w(1, 1, 1)
            load_w(2, 2, 2)
            load_w(3, 4, 4)
            load_x0_piece(1)
            load_w(4, 8, 4)
            load_w(5, 12, 4)
            load_x0_piece(2)
            load_w(6, 16, 4)
            load_w(7, 20, 4)
            load_x0_piece(3)
            load_w(8, 24, 4)
            load_w(9, 28, 4)
            xb_pre = [xb0]

            biasw = cst.tile([P, O_LOC], DT.float32)
            nc.gpsimd.dma_start(out=biasw[:],
                                in_=biasb[0, :].partition_broadcast(P))

            # ---- HAM warm-up: ~3.5µs of dummy matmuls while the first
            # x/w DMAs land, so the PE clock-gate is released before the
            # real stream starts (the PE is idle here regardless) ----
            warm = cst.tile([P, 352], DT.bfloat16)
            nc.vector.memset(warm[:], 0.0)
            warm_ps = psp.tile([P, 352], DT.float32, tag="ps")
            for _ in range(26):
                nc.tensor.matmul(warm_ps[:], warm[:, :P], warm[:],
                                 start=True, stop=True)

            def mm_block(sp, xb):
                for half in range(2):
                    s0 = sp * SP + half * P
                    sl = slice(half * P, (half + 1) * P)
                    ps_ts = [psp.tile([P, osz], DT.float32, tag="ps",
                                      name=f"ps_{sp}_{half}_{oi}")
                             for oi, (o0, osz) in enumerate(O_CHUNKS)]
                    for kt in range(KT):
                        w_t, row = wf[kt]
                        for oi, (o0, osz) in enumerate(O_CHUNKS):
                            nc.tensor.matmul(
                                ps_ts[oi][:], xb[:, kt, sl],
                                w_t[:, row, o0:o0 + osz],
                                start=(kt == 0), stop=(kt == KT - 1))
                    out_t = op.tile([P, O_LOC], DT.float32, tag="out")
                    # alternate store ring: one queue's bandwidth is not
                    # enough to also absorb contention spikes
                    eng = nc.scalar if half == 0 else nc.gpsimd
                    last = sp == NSP - 1
                    for oi, (o0, osz) in enumerate(O_CHUNKS):
                        nc.vector.tensor_tensor(out_t[:, o0:o0 + osz],
                                                ps_ts[oi][:],
                                                biasw[:, o0:o0 + osz],
                                                AluOpType.add)
                        if last:
                            # chunked stores at the tail start earlier and
                            # shrink the end-of-kernel drain
                            nc.scalar.dma_start(out=y[s0:s0 + P, o0:o0 + osz],
                                                in_=out_t[:, o0:o0 + osz])
                    if not last:
                        eng.dma_start(out=y[s0:s0 + P, :], in_=out_t[:])

            for sp in range(NSP):
                xb_cur = xb_pre.pop(0)
                if sp + 1 < NSP:
                    xb_pre.append(load_x(sp + 1))
                mm_block(sp, xb_cur)

    _dedupe_ldweights(nc)
    if STRIP_MM_UPDATES:
        _strip_mm_updates(nc)
    nc.compile()
    return nc


def _strip_mm_updates(nc):
    """Drop the per-matmul semaphore increment from non-stop matmuls
    (only accumulation-group-final matmuls gate any consumer) and remap
    every wait threshold on the affected semaphores. EVT_SEM register
    writes serialize on the PE sequencer, so 6k of them is real time."""
    fn = nc.m.functions[0]
    insts = []
    for blk in fn.blocks:
        insts.extend(blk.instructions)

    # pass 0: only touch semaphores whose every producer is a PE
    # engine instruction (mixed-producer sems can't be recounted here)
    pe_only = {}
    for inst in insts:
        si = inst.sync_info
        if si is None or not si.on_update:
            continue
        is_pe = getattr(inst, "engine", None) == mybir.EngineType.PE
        for u in si.on_update:
            if u.sync_type == "semaphore":
                pe_only[u.id] = pe_only.get(u.id, True) and is_pe

    # pass 1: per-sem cumulative counts at each update point
    cum = {}          # sem id -> [old_cum, new_cum]
    points = {}       # sem id -> list[(old_cum_after, new_cum_after)]
    for inst in insts:
        si = inst.sync_info
        if si is None or not si.on_update:
            continue
        if getattr(inst, "engine", None) != mybir.EngineType.PE:
            continue
        ups = si.on_update
        droppable = (
            isinstance(inst, mybir.InstMatmult)
            and not inst.stop_tensor_calc
            and len(ups) == 1
            and ups[0].sync_type == "semaphore"
            and ups[0].update_mode == "sem-inc"
            and ups[0].update_value == 1
            and pe_only.get(ups[0].id, False))
        for u in ups:
            if (u.sync_type != "semaphore" or u.update_mode != "sem-inc"
                    or not pe_only.get(u.id)):
                continue
            oc, nc_ = cum.get(u.id, (0, 0))
            oc += u.update_value
            if not droppable:
                nc_ += u.update_value
                points.setdefault(u.id, []).append((oc, nc_))
            cum[u.id] = (oc, nc_)
        if droppable:
            si.on_update = []

    # pass 2: remap waits (round up to the next kept point)
    for inst in insts:
        si = inst.sync_info
        if si is None or not si.on_wait:
            continue
        for w in si.on_wait:
            if (w.sync_type == "semaphore" and w.id in points
                    and w.wait_mode == "sem-ge-imm" and w.wait_value > 0):
                pts = points[w.id]
                v = w.wait_value
                new_v = None
                for oc, nc_ in pts:
                    if oc >= v:
                        new_v = nc_
                        break
                assert new_v is not None, (w.id, v, pts[-1])
                w.wait_value = new_v


def _dedupe_ldweights(nc):
    """Drop InstLdweights that reload the stationary operand already in
    the PE array (same weights AP as the previous load, no semaphores).
    Tile legalization pairs every matmul with its own Ldweights; the 3
    matmuls sharing one x-stationary per k-tile only need the first."""
    fn = nc.m.functions[0]
    ldw_t = mybir.InstLdweights
    mm_t = mybir.InstMatmult
    sem_t = mybir.InstEventSemaphore
    dropped = 0
    for blk in fn.blocks:
        insts = blk.instructions
        keep = []
        last_key = None
        for inst in insts:
            eng = getattr(inst, "engine", None)
            if eng != mybir.EngineType.PE:
                keep.append(inst)
                continue
            if isinstance(inst, ldw_t):
                si = inst.sync_info
                clean = si is None or (not si.on_wait and not si.on_update)
                key = (str(inst.ins[0]), str(inst.perf_mode),
                       str(inst.is_transpose), str(inst.tile_position))
                if clean and key == last_key:
                    dropped += 1
                    continue
                last_key = key
                keep.append(inst)
            elif isinstance(inst, (mm_t, sem_t)):
                keep.append(inst)
            else:
                last_key = None
                keep.append(inst)
        if len(keep) != len(insts):
            insts[:] = keep
    return dropped


def _get_nc():
    if 'nc' not in _CACHE:
        _CACHE['nc'] = _build()
    return _CACHE['nc']


def make_in_maps(x, weight_q, absmax, bias):
    x = np.asarray(x, dtype=np.float32)
    weight_q = np.asarray(weight_q)
    absmax = np.asarray(absmax, dtype=np.float32)
    bias = np.asarray(bias, dtype=np.float32)
    bf16 = ml_dtypes.bfloat16

    # xH[p, ((sp*KT)+kt)*SP + s] = x[sp*SP+s, kt*128+p] — every x-tile
    # DMA reads a fully contiguous per-partition span
    xH = np.ascontiguousarray(
        x.reshape(NSP, SP, KT, P).transpose(3, 0, 2, 1)
        .reshape(P, NSP * KT * SP).astype(bf16))
    # Blockwise NF4 dequant on host: code lookup * per-block absmax
    codes = NF4[weight_q]                                    # [O, I] f32
    w = codes.reshape(OUT_F, IN_F // 64, 64) * absmax[:, :, None]
    w = w.reshape(OUT_F, IN_F)
    in_maps = []
    for c in range(N_CORES):
        sl = slice(c * O_LOC, (c + 1) * O_LOC)
        # [O_LOC, I] -> [I, O_LOC] -> [KT, P, O_LOC] -> [P, KT*O_LOC]
        w_c = np.ascontiguousarray(
            w[sl].T.reshape(KT, P, O_LOC).transpose(1, 0, 2)
            .reshape(P, KT * O_LOC).astype(bf16))
        biasb_c = np.ascontiguousarray(bias[sl].astype(bf16).reshape(1, O_LOC))
        in_maps.append({"xH": xH, "wd": w_c, "biasb": biasb_c})
    return in_maps


def kernel(x, weight_q, absmax, bias):
    nc = _get_nc()
    in_maps = make_in_maps(x, weight_q, absmax, bias)
    res = run_bass_kernel_spmd(nc, in_maps, core_ids=list(range(N_CORES)))
    y = np.concatenate([res.results[c]["y"] for c in range(N_CORES)], axis=1)
    return np.ascontiguousarray(y.reshape(4, 2048, OUT_F))


# revision 29
# speedup vs baseline: 1.4680x; 1.4680x over previous
"""Trainium2 Bass kernel: NF4 (bitsandbytes-style) dequant + linear.

y = x @ dequant(weight_q, absmax).T + bias

x:        [4, 2048, 4096] f32
weight_q: [11008, 4096] int32 (values 0..15, NF4 codes)
absmax:   [11008, 64] f32 (per-64-block scales)
bias:     [11008] f32
out:      [4, 2048, 11008] f32

Sharding: column-parallel over out_features across 8 cores (1376 each).
Compute in bf16 with f32 PSUM accumulation. Host prep dequantizes the
weight (16-entry NF4 code lookup × per-64-block absmax) to bf16 and lays
it out contraction-major [128, KT*O_LOC]; x is host-blocked to
[128, NSP*KT*SP] bf16 so the contraction dim is on SBUF partitions and
every x-tile DMA reads a fully contiguous 16KB span per partition.

Device schedule: weight chunks stream in interleaved with x(0) in the
order the first token tile consumes them (graded sizes, small first),
so the PE starts within ~2µs and never waits on weight delivery after
the first token tile; ~3.5µs of dummy matmuls warm the PE clock-gate
(HAM) while those DMAs land. The matmul loop is kt-outer /
o-chunk-inner: each stationary (x) tile feeds 3 matmuls (512+512+352 =
1376 moving columns per 128-token stationary load). PSUM accumulates
over the 32 k-tiles; eviction adds the bias on DVE and stores ride
alternating DMA rings. Two post-schedule BIR passes cut PE sequencer
overhead: redundant Ldweights reloads of the same stationary are
deleted (2 of every 3), and per-matmul semaphore increments are kept
only on accumulation-group-final matmuls (waits remapped to match).
"""

import numpy as np
import ml_dtypes

import concourse.bacc as bacc
import concourse.mybir as mybir
import concourse.tile as tile
from concourse.alu_op_type import AluOpType
from concourse.bass_utils import run_bass_kernel_spmd

DT = mybir.dt

NF4 = np.array([
    -1.0, -0.6961928009986877, -0.5250730514526367, -0.39491748809814453,
    -0.28444138169288635, -0.18477343022823334, -0.09105003625154495, 0.0,
    0.07958029955625534, 0.16093020141124725, 0.24611230194568634,
    0.33791524171829224, 0.44070982933044434, 0.5626170039176941,
    0.7229568362236023, 1.0], dtype=np.float32)

P = 128
IN_F = 4096
OUT_F = 11008
N_CORES = 8
O_LOC = OUT_F // N_CORES          # 1376 out features per core
S_TOT = 4 * 2048                  # 8192 tokens
KT = IN_F // P                    # 32 contraction tiles
SP = 256                          # tokens per x macro tile (2 psum tiles)
NSP = S_TOT // SP                 # 32 x macro tiles
O_CHUNKS = [(0, 512), (512, 512), (1024, 352)]
STRIP_MM_UPDATES = True

_CACHE = {}


def _build():
    nc = bacc.Bacc()
    # x host-blocked so every x-tile DMA is fully contiguous per
    # partition: xH[p, sp, kt, s] = x[sp*SP+s, kt*128+p]
    xH = nc.dram_tensor("xH", [P, NSP * KT * SP], DT.bfloat16,
                        kind="ExternalInput")
    w_d = nc.dram_tensor("wd", [P, KT * O_LOC], DT.bfloat16,
                         kind="ExternalInput")
    biasb = nc.dram_tensor("biasb", [1, O_LOC], DT.bfloat16,
                           kind="ExternalInput")
    y = nc.dram_tensor("y", [S_TOT, O_LOC], DT.float32, kind="ExternalOutput")

    with tile.TileContext(nc) as tc:
        with (
            tc.tile_pool(name="w", bufs=1) as wpool,
            tc.tile_pool(name="x", bufs=3) as xp,
            tc.tile_pool(name="o", bufs=4) as op,
            tc.tile_pool(name="ps", bufs=8, space="PSUM") as psp,
            tc.tile_pool(name="c", bufs=1) as cst,
        ):
            def load_x(sp):
                c0 = sp * KT * SP
                xb = xp.tile([P, KT, SP], DT.bfloat16, tag="xb", name="xb")
                nc.sync.dma_start(
                    out=xb[:], in_=xH[:, c0:c0 + KT * SP]
                    .rearrange("p (k s) -> p k s", k=KT))
                return xb

            wf = {}           # kt -> (tile, row within tile)

            def load_w(g, k0, kg, eng=None):
                w_t = wpool.tile([P, kg, O_LOC], DT.bfloat16, tag=f"wf_{g}")
                (eng or nc.sync).dma_start(
                    out=w_t[:],
                    in_=w_d[:, k0 * O_LOC:(k0 + kg) * O_LOC]
                        .rearrange("p (k o) -> p k o", k=kg))
                for k in range(kg):
                    wf[k0 + k] = (w_t, k)

            # ---- startup: x(0) in 4 pieces interleaved with graded w
            # chunks, ordered so neither PE operand stream starves ----
            xb0 = xp.tile([P, KT, SP], DT.bfloat16, tag="xb", name="xb")

            def load_x0_piece(g):
                nc.sync.dma_start(
                    out=xb0[:, g * 8:(g + 1) * 8, :],
                    in_=xH[:, g * 8 * SP:(g + 1) * 8 * SP]
                        .rearrange("p (k s) -> p k s", k=8))

            load_x0_piece(0)
            load_w(0, 0, 1)
            load_w(1, 1, 1)
            load_w(2, 2, 2)
            load_w(3, 4, 4)
            load_x0_piece(1)
            load_w(4, 8, 4)
            load_w(5, 12, 4)
            load_x0_piece(2)
            load_w(6, 16, 4)
            load_w(7, 20, 4)
            load_x0_piece(3)
            load_w(8, 24, 4)
            load_w(9, 28, 4)
            xb_pre = [xb0]

            biasw = cst.tile([P, O_LOC], DT.float32)
            nc.gpsimd.dma_start(out=biasw[:],
                                in_=biasb[0, :].partition_broadcast(P))

            # ---- HAM warm-up: ~3.5µs of dummy matmuls while the first
            # x/w DMAs land, so the PE clock-gate is released before the
            # real stream starts (the PE is idle here regardless) ----
            warm = cst.tile([P, 352], DT.bfloat16)
            nc.vector.memset(warm[:], 0.0)
            warm_ps = psp.tile([P, 352], DT.float32, tag="ps")
            for _ in range(26):
                nc.tensor.matmul(warm_ps[:], warm[:, :P], warm[:],
                                 start=True, stop=True)

            def mm_block(sp, xb):
                for half in range(2):
                    s0 = sp * SP + half * P
                    sl = slice(half * P, (half + 1) * P)
                    ps_ts = [psp.tile([P, osz], DT.float32, tag="ps",
                                      name=f"ps_{sp}_{half}_{oi}")
                             for oi, (o0, osz) in enumerate(O_CHUNKS)]
                    for kt in range(KT):
                        w_t, row = wf[kt]
                        for oi, (o0, osz) in enumerate(O_CHUNKS):
                            nc.tensor.matmul(
                                ps_ts[oi][:], xb[:, kt, sl],
                                w_t[:, row, o0:o0 + osz],
                                start=(kt == 0), stop=(kt == KT - 1))
                    out_t = op.tile([P, O_LOC], DT.float32, tag="out")
                    # alternate store ring: one queue's bandwidth is not
                    # enough to also absorb contention spikes
                    eng = nc.scalar if half == 0 else nc.gpsimd
                    last = sp == NSP - 1
                    for oi, (o0, osz) in enumerate(O_CHUNKS):
                        nc.vector.tensor_tensor(out_t[:, o0:o0 + osz],
                                                ps_ts[oi][:],
                                                biasw[:, o0:o0 + osz],
                                                AluOpType.add)
                        if last:
                            # chunked stores at the tail start earlier and
                            # shrink the end-of-kernel drain
                            nc.scalar.dma_start(out=y[s0:s0 + P, o0:o0 + osz],
                                                in_=out_t[:, o0:o0 + osz])
                    if not last:
                        eng.dma_start(out=y[s0:s0 + P, :], in_=out_t[:])

            for sp in range(NSP):
                xb_cur = xb_pre.pop(0)
                if sp + 1 < NSP:
                    xb_pre.append(load_x(sp + 1))
                mm_block(sp, xb_cur)

    _dedupe_ldweights(nc)
    if STRIP_MM_UPDATES:
        _strip_mm_updates(nc)
    nc.compile()
    return nc


def _strip_mm_updates(nc):
    """Drop the per-matmul semaphore increment from non-stop matmuls
    (only accumulation-group-final matmuls gate any consumer) and remap
    every wait threshold on the affected semaphores. EVT_SEM register
    writes serialize on the PE sequencer, so 6k of them is real time."""
    fn = nc.m.functions[0]
    insts = []
    for blk in fn.blocks:
        insts.extend(blk.instructions)

    # pass 0: only touch semaphores whose every producer is a PE
    # engine instruction (mixed-producer sems can't be recounted here)
    pe_only = {}
    for inst in insts:
        si = inst.sync_info
        if si is None or not si.on_update:
            continue
        is_pe = getattr(inst, "engine", None) == mybir.EngineType.PE
        for u in si.on_update:
            if u.sync_type == "semaphore":
                pe_only[u.id] = pe_only.get(u.id, True) and is_pe

    # pass 1: per-sem cumulative counts at each update point
    cum = {}          # sem id -> [old_cum, new_cum]
    points = {}       # sem id -> list[(old_cum_after, new_cum_after)]
    for inst in insts:
        si = inst.sync_info
        if si is None or not si.on_update:
            continue
        if getattr(inst, "engine", None) != mybir.EngineType.PE:
            continue
        ups = si.on_update
        droppable = (
            isinstance(inst, mybir.InstMatmult)
            and not inst.stop_tensor_calc
            and len(ups) == 1
            and ups[0].sync_type == "semaphore"
            and ups[0].update_mode == "sem-inc"
            and ups[0].update_value == 1
            and pe_only.get(ups[0].id, False))
        for u in ups:
            if (u.sync_type != "semaphore" or u.update_mode != "sem-inc"
                    or not pe_only.get(u.id)):
                continue
            oc, nc_ = cum.get(u.id, (0, 0))
            oc += u.update_value
            if not droppable:
                nc_ += u.update_value
                points.setdefault(u.id, []).append((oc, nc_))
            cum[u.id] = (oc, nc_)
        if droppable:
            si.on_update = []

    # pass 2: remap waits (round up to the next kept point)
    for inst in insts:
        si = inst.sync_info
        if si is None or not si.on_wait:
            continue
        for w in si.on_wait:
            if (w.sync_type == "semaphore" and w.id in points
                    and w.wait_mode == "sem-ge-imm" and w.wait_value > 0):
                pts = points[w.id]
                v = w.wait_value
                new_v = None
                for oc, nc_ in pts:
                    if oc >= v:
                        new_v = nc_
                        break
                assert new_v is not None, (w.id, v, pts[-1])
                w.wait_value = new_v


def _dedupe_ldweights(nc):
    """Drop InstLdweights that reload the stationary operand already in
    the PE array (same weights AP as the previous load, no semaphores).
    Tile legalization pairs every matmul with its own Ldweights; the 3
    matmuls sharing one x-stationary per k-tile only need the first."""
    fn = nc.m.functions[0]
    ldw_t = mybir.InstLdweights
    mm_t = mybir.InstMatmult
    sem_t = mybir.InstEventSemaphore
    dropped = 0
    for blk in fn.blocks:
        insts = blk.instructions
        keep = []
        last_key = None
        for inst in insts:
            eng = getattr(inst, "engine", None)
            if eng != mybir.EngineType.PE:
                keep.append(inst)
                continue
            if isinstance(inst, ldw_t):
                si = inst.sync_info
                clean = si is None or (not si.on_wait and not si.on_update)
                key = (str(inst.ins[0]), str(inst.perf_mode),
                       str(inst.is_transpose), str(inst.tile_position))
                if clean and key == last_key:
                    dropped += 1
                    continue
                last_key = key
                keep.append(inst)
            elif isinstance(inst, (mm_t, sem_t)):
                keep.append(inst)
            else:
                last_key = None
                keep.append(inst)
        if len(keep) != len(insts):
            insts[:] = keep
    return dropped


def _get_nc():
    if 'nc' not in _CACHE:
        _CACHE['nc'] = _build()
    return _CACHE['nc']


def make_in_maps(x, weight_q, absmax, bias):
    x = np.asarray(x, dtype=np.float32)
    weight_q = np.asarray(weight_q)
    absmax = np.asarray(absmax, dtype=np.float32)
    bias = np.asarray(bias, dtype=np.float32)
    bf16 = ml_dtypes.bfloat16

    # xH[p, ((sp*KT)+kt)*SP + s] = x[sp*SP+s, kt*128+p] — every x-tile
    # DMA reads a fully contiguous per-partition span
    xH = np.ascontiguousarray(
        x.reshape(NSP, SP, KT, P).transpose(3, 0, 2, 1)
        .reshape(P, NSP * KT * SP).astype(bf16))
    # Blockwise NF4 dequant on host: code lookup * per-block absmax
    codes = NF4[weight_q]                                    # [O, I] f32
    w = codes.reshape(OUT_F, IN_F // 64, 64) * absmax[:, :, None]
    w = w.reshape(OUT_F, IN_F)
    in_maps = []
    for c in range(N_CORES):
        sl = slice(c * O_LOC, (c + 1) * O_LOC)
        # [O_LOC, I] -> [I, O_LOC] -> [KT, P, O_LOC] -> [P, KT*O_LOC]
        w_c = np.ascontiguousarray(
            w[sl].T.reshape(KT, P, O_LOC).transpose(1, 0, 2)
            .reshape(P, KT * O_LOC).astype(bf16))
        biasb_c = np.ascontiguousarray(bias[sl].astype(bf16).reshape(1, O_LOC))
        in_maps.append({"xH": xH, "wd": w_c, "biasb": biasb_c})
    return in_maps


def kernel(x, weight_q, absmax, bias):
    nc = _get_nc()
    in_maps = make_in_maps(x, weight_q, absmax, bias)
    res = run_bass_kernel_spmd(nc, in_maps, core_ids=list(range(N_CORES)))
    y = np.concatenate([res.results[c]["y"] for c in range(N_CORES)], axis=1)
    return np.ascontiguousarray(y.reshape(4, 2048, OUT_F))
